# revision 15
# baseline (speedup 1.0000x reference)
"""MSE + SSIM combined loss on Trainium2, data-parallel over 8 NeuronCores.

Reference computes, over [64,3,512,512] f32 inputs:
    loss = 0.7*mean((x-y)^2) + 0.3*(1 - mean(ssim_map(x, y)))
with an 11x11 gaussian (sigma=1.5) depthwise conv, zero-padded (pad=5).

Per core (8 images = 24 channel-images of [512,512]):
  - cast-during-DMA loads: xb, yb [128, 4*512] bf16 (partition p holds rows
    {p, 128+p, 256+p, 384+p})
  - prep on [128, 2048] tiles: x2,y2 = Square on ACT; s = x2+y2 and
    xy = xb*yb via DVE scalar_tensor_tensor, whose accum_out emits the
    per-partition sums for the MSE for free
  - separable gaussian conv as two banded matmul passes on TensorE:
      d1 (h-conv, transposing): ps1[w_blk, h] += X[h'_blk, w_blk]^T G[h'_blk, band]
      d2 (w-conv): M[wb, h] += G[wt, wb]^T o1[wt, h]; x and xy fields use 2G
        so the PSUM results are M1=2*mu1, XY=2*conv(xy) directly
  - PSUM evacuations all on ACT with folded scale/bias:
      a1=M1, a2=M2, q1=Square(0.5*M1)=mu1^2, q2=mu2^2, xc=XY+C2, sc=S+C1+C2
  - ssim elementwise per image on [128, 2048] bf16 tiles on DVE:
      P2=a1*a2 (=2 mu1 mu2); num=(P2+C1)*(xc-P2); den1=(q1+C1)+q2;
      den=den1*(sc-den1); rden via the 1-op reciprocal approximation;
      ssim=num*rden summed via scalar_tensor_tensor accum_out
  - host combines the [128, 3*NIMG] per-partition partial sums
"""

import numpy as np
from contextlib import ExitStack

import concourse.bass as bass
import concourse.bacc as bacc
import concourse.mybir as mybir
from concourse import tile
from concourse.bass_utils import run_bass_kernel_spmd

F32 = mybir.dt.float32
BF16 = mybir.dt.bfloat16
AF = mybir.ActivationFunctionType
ALU = mybir.AluOpType

# ---- problem constants (hardcoded; kernel.py must be self-contained) ----
WIN = 11
SIGMA = 1.5
PAD = WIN // 2
DATA_RANGE = 2.0
MSE_W = 0.7
SSIM_W = 0.3
C1 = (0.01 * DATA_RANGE) ** 2
C2 = (0.03 * DATA_RANGE) ** 2

B, C, H, W = 64, 3, 512, 512
NCORES = 8
NIMG = (B // NCORES) * C      # 24 channel-images per core
NT = H // 128                 # 4 tiles per image dim
FD = NT * W                   # 2048 free-dim for per-image tiles


def _gauss1d():
    coords = np.arange(WIN, dtype=np.float64) - (WIN - 1) / 2.0
    g = np.exp(-(coords ** 2) / (2.0 * SIGMA ** 2))
    return (g / g.sum()).astype(np.float32)


def _band_matrix():
    """G[i, j] = g1d[j - i + PAD] for |j-i|<=PAD else 0  (512x512 f32)."""
    g = _gauss1d()
    G = np.zeros((H, H), dtype=np.float32)
    for d in range(-PAD, PAD + 1):
        np.fill_diagonal(G[max(0, -d):, max(0, d):], g[d + PAD])
    return G


def _band(k):
    """Nonzero output-column range of G rows [128k, 128k+128)."""
    return max(0, 128 * k - PAD), min(H, 128 * (k + 1) + PAD)


def build_nc(sim_compat=False):
    nc = bacc.Bacc("TRN2")
    x_ext = nc.declare_dram_parameter("x", [NIMG, NT, 128, W], F32, isOutput=False)
    y_ext = nc.declare_dram_parameter("y", [NIMG, NT, 128, W], F32, isOutput=False)
    g_ext = nc.declare_dram_parameter("g", [NT, 128, H], F32, isOutput=False)
    g2_ext = nc.declare_dram_parameter("g2", [NT, 128, H], F32, isOutput=False)
    # per-partition partial sums: [0:N]=s, [N:2N]=xy, [2N:3N]=ssim
    out_ext = nc.declare_dram_parameter("out", [128, 3 * NIMG], F32, isOutput=True)

    with ExitStack() as ctx:
        tc = ctx.enter_context(tile.TileContext(nc))
        const_pool = ctx.enter_context(tc.tile_pool(name="const", bufs=1))
        in_pool = ctx.enter_context(tc.tile_pool(name="inp", bufs=3))
        fld_pool = ctx.enter_context(tc.tile_pool(name="fld", bufs=3))
        o1_pool = ctx.enter_context(tc.tile_pool(name="o1", bufs=2))
        ev_pool = ctx.enter_context(tc.tile_pool(name="ev", bufs=2))
        ew_pool = ctx.enter_context(tc.tile_pool(name="ew", bufs=1))
        ps1_pool = ctx.enter_context(tc.tile_pool(name="ps1", bufs=2, space="PSUM"))
        ps2_pool = ctx.enter_context(tc.tile_pool(name="ps2", bufs=1, space="PSUM"))

        # ---- constants: G blocks as bf16 (cast during DMA) ----
        Gsb, G2sb = [], []
        for k in range(NT):
            gk = const_pool.tile([128, H], BF16, tag=f"g{k}")
            nc.gpsimd.dma_start(gk[:], g_ext[k])
            g2k = const_pool.tile([128, H], BF16, tag=f"g2{k}")
            nc.gpsimd.dma_start(g2k[:], g2_ext[k])
            Gsb.append(gk)
            G2sb.append(g2k)

        # ---- per-partition accumulators (written column-per-image) ----
        acc = const_pool.tile([128, 3 * NIMG], F32, tag="acc")

        for i in range(NIMG):
            # ---- load (cast f32 -> bf16 during DMA) ----
            xb = in_pool.tile([128, NT, W], BF16, tag="xb")
            nc.gpsimd.dma_start(xb[:], x_ext[i].rearrange("t p w -> p t w"))
            yb = in_pool.tile([128, NT, W], BF16, tag="yb")
            nc.gpsimd.dma_start(yb[:], y_ext[i].rearrange("t p w -> p t w"))
            xb = xb.rearrange("p t w -> p (t w)")
            yb = yb.rearrange("p t w -> p (t w)")

            # ---- field prep ----
            x2 = fld_pool.tile([128, FD], BF16, tag="x2")
            nc.scalar.activation(x2[:], xb, AF.Square)
            y2 = fld_pool.tile([128, FD], BF16, tag="y2")
            nc.scalar.activation(y2[:], yb, AF.Square)
            s = fld_pool.tile([128, FD], BF16, tag="s")
            nc.vector.scalar_tensor_tensor(
                s[:], x2[:], 0.0, y2[:], ALU.add, ALU.add,
                accum_out=acc[:, i:i + 1])
            xy = fld_pool.tile([128, FD], BF16, tag="xy")
            nc.vector.scalar_tensor_tensor(
                xy[:], xb, 0.0, yb, ALU.add, ALU.mult,
                accum_out=acc[:, NIMG + i:NIMG + i + 1])

            fields = [xb, yb, s[:], xy[:]]

            # ---- d1: h-conv, transposing.  o1[f][:, 512wb:] = [w_blk, h] ----
            o1 = []
            for f in range(4):
                o1f = o1_pool.tile([128, FD], BF16, tag=f"o1_{f}")
                o1.append(o1f)
                for wp in range(2):
                    ps1 = ps1_pool.tile([128, 2 * H], F32, tag="ps1")
                    for half in range(2):
                        wb = 2 * wp + half
                        for k in range(NT):
                            # sim models has_written per-instruction; stream
                            # full width on the start matmul there only.  HW
                            # tracks has_written per element, so bands suffice.
                            lo, hi = (0, H) if (k == 0 and sim_compat) else _band(k)
                            nc.tensor.matmul(
                                ps1[:, H * half + lo:H * half + hi],
                                lhsT=fields[f][:, W * k + 128 * wb:W * k + 128 * (wb + 1)],
                                rhs=Gsb[k][:, lo:hi],
                                start=(k == 0), stop=(k == NT - 1),
                                skip_group_check=True)
                    nc.scalar.copy(o1f[:, 2 * W * wp:2 * W * (wp + 1)], ps1[:])

            # ---- d2: w-conv + ACT evacuations with folded scale/bias ----
            a1 = ev_pool.tile([128, FD], BF16, tag="a1")
            a2 = ev_pool.tile([128, FD], BF16, tag="a2")
            q1 = ev_pool.tile([128, FD], BF16, tag="q1")
            q2 = ev_pool.tile([128, FD], BF16, tag="q2")
            xc = ev_pool.tile([128, FD], BF16, tag="xc")
            sc = ev_pool.tile([128, FD], BF16, tag="sc")
            for wb in range(NT):
                parts = []
                if wb > 0:
                    parts.append((wb - 1, 64, 128))
                parts.append((wb, 0, 128))
                if wb < NT - 1:
                    parts.append((wb + 1, 0, 32))
                ps2 = []
                for f in range(4):
                    p = ps2_pool.tile([128, H], F32, tag=f"ps2_{f}")
                    gmat = G2sb if f in (0, 3) else Gsb
                    for j, (wt, r0, r1) in enumerate(parts):
                        nc.tensor.matmul(
                            p[:, :],
                            lhsT=gmat[wt][r0:r1, 128 * wb:128 * (wb + 1)],
                            rhs=o1[f][r0:r1, W * wt:W * (wt + 1)],
                            start=(j == 0), stop=(j == len(parts) - 1))
                    ps2.append(p)
                M1, M2, S, XY = ps2
                sl = slice(W * wb, W * (wb + 1))
                nc.scalar.copy(a1[:, sl], M1[:])
                nc.scalar.copy(a2[:, sl], M2[:])
                nc.scalar.activation(q1[:, sl], M1[:], AF.Square, scale=0.5)
                nc.scalar.activation(q2[:, sl], M2[:], AF.Square)
                nc.vector.tensor_scalar_add(xc[:, sl], XY[:], C2)
                nc.vector.tensor_scalar_add(sc[:, sl], S[:], C1 + C2)

            # ---- ssim elementwise on [128, 2048] ----
            P2 = ew_pool.tile([128, FD], BF16, tag="P2")
            nc.vector.tensor_tensor(P2[:], a1[:], a2[:], ALU.mult)
            n2 = ew_pool.tile([128, FD], BF16, tag="n2")
            nc.vector.tensor_tensor(n2[:], xc[:], P2[:], ALU.subtract)
            num = ew_pool.tile([128, FD], BF16, tag="num")
            nc.vector.scalar_tensor_tensor(
                num[:], P2[:], C1, n2[:], ALU.add, ALU.mult)
            den1 = ew_pool.tile([128, FD], BF16, tag="den1")
            nc.vector.scalar_tensor_tensor(
                den1[:], q1[:], C1, q2[:], ALU.add, ALU.add)
            den2 = ew_pool.tile([128, FD], BF16, tag="den2")
            nc.vector.tensor_tensor(den2[:], sc[:], den1[:], ALU.subtract)
            den = ew_pool.tile([128, FD], F32, tag="den")
            nc.vector.tensor_tensor(den[:], den1[:], den2[:], ALU.mult)
            rden = ew_pool.tile([128, FD], F32, tag="rden")
            nc.vector.reciprocal_approx_fast(rden[:], den[:])
            scr = ew_pool.tile([128, FD], BF16, tag="scr")
            nc.vector.scalar_tensor_tensor(
                scr[:], num[:], 0.0, rden[:], ALU.add, ALU.mult,
                accum_out=acc[:, 2 * NIMG + i:2 * NIMG + i + 1])

        nc.gpsimd.dma_start(out_ext[:, :], acc[:])
    nc.compile()
    return nc


_NC_CACHE = None


def _get_nc():
    global _NC_CACHE
    if _NC_CACHE is None:
        _NC_CACHE = build_nc()
    return _NC_CACHE


last_exec_time_ns = None


def kernel(recon, original, _trace=False):
    global last_exec_time_ns
    recon = np.ascontiguousarray(np.asarray(recon, dtype=np.float32))
    original = np.ascontiguousarray(np.asarray(original, dtype=np.float32))
    G = _band_matrix()
    G4 = G.reshape(NT, 128, H)
    G24 = (2.0 * G).reshape(NT, 128, H)

    per = B // NCORES
    in_maps = []
    for c in range(NCORES):
        in_maps.append({
            "x": recon[c * per:(c + 1) * per].reshape(NIMG, NT, 128, W),
            "y": original[c * per:(c + 1) * per].reshape(NIMG, NT, 128, W),
            "g": G4,
            "g2": G24,
        })

    nc = _get_nc()
    res = run_bass_kernel_spmd(nc, in_maps, list(range(NCORES)), trace=_trace)
    last_exec_time_ns = res.exec_time_ns

    n_total = float(B * C * H * W)
    s_ssim = s_s = s_xy = 0.0
    for c in range(NCORES):
        out = np.asarray(res.results[c]["out"], dtype=np.float64)
        s_s += out[:, :NIMG].sum()
        s_xy += out[:, NIMG:2 * NIMG].sum()
        s_ssim += out[:, 2 * NIMG:].sum()

    mse = (s_s - 2.0 * s_xy) / n_total
    ssim_mean = s_ssim / n_total
    loss = MSE_W * mse + SSIM_W * (1.0 - ssim_mean)
    return np.float32(loss)
